# revision 45
# baseline (speedup 1.0000x reference)
"""Banded soft-DTW loss (normalize=True) Trainium2 Bass kernel.

Problem: x, y [32, 512, 4] f32 -> loss [32] f32
  loss = softdtw(x,y) - 0.5*(softdtw(x,x) + softdtw(y,y)), gamma=2, band=50.

Strategy — forward/backward split (halves the serial DP chain):
  * The soft-DTW path-sum factors at the middle cut:
      S_total = sum_j F[255,j] * (B[256,j] + B[256,j+1])
    where F is the forward exp-space DP after rows 0..255 and B the
    backward DP, which equals the forward DP of the REVERSED sequences.
  * Cores 0-3 run forward halves, cores 4-7 run the same program on
    host-reversed inputs. Each core: 24 DP problems (8 batches x
    {xy,xx,yy}) x 256 rows, batched across 24 SBUF partitions.
  * Cost matrices: one K=5 augmented matmul per (problem, 128-row chunk)
    computes a.b - |b|^2/2 over a 228-wide j-window; the -|a|^2/2 term is
    applied as the exp's per-partition bias (from tiny transposed
    matmuls); (xy, xx) problems share one PSUM tile and one ACT exp.
    DRAM round-trip extracts the 101-wide diagonal band per row, written
    in latency-tiered slabs and gathered into 32-row groups that carry a
    ones-prefix for the DP scan below.
  * DP: ONE fused 202-element tensor_tensor_scan per row (emitted at the
    ISA level; the HW chains scan state across access-pattern dims):
    per t, element (t,0) adds S_prev[t] (times the ones-prefix) and
    (t,1) adds S_prev[t+1] and multiplies by E_i[t] — the full row
    recurrence S_i[t] = E_i[t]*(S_i[t-1] + S_{i-1}[t] + S_{i-1}[t+1])
    with no separate add op. S rows live in doubled layout (odd slots);
    rows < BW start at the band edge. Rescale by the diagonal cell every
    RS rows (log accumulated, applied to odd slots only).
  * Each core outputs its boundary row + log-rescale sum; the host does
    the tiny O(B*W) combine.
"""
import os
import sys
from contextlib import ExitStack

import numpy as np

for _p in ("/opt/trn_rl_repo", "/root/.axon_site/_ro/trn_rl_repo"):
    if os.path.isdir(_p) and _p not in sys.path:
        sys.path.append(_p)

import concourse.bass as bass
import concourse.bacc as bacc
import concourse.mybir as mybir
import concourse.tile as tile
from concourse.bass_utils import run_bass_kernel_spmd

F32 = mybir.dt.float32
BF16 = mybir.dt.bfloat16
ALU = mybir.AluOpType
ACTF = mybir.ActivationFunctionType

N = 512            # full sequence length
HN = 256           # rows per half (per core)
DIM = 4
NBAT = 8           # batch elements per core
NSEQ = 16          # sequences per core (8 x + 8 y)
NPROB = 24         # DP problems per core (xy, xx, yy for 8 batches)
NCORE = 8
BW = 50            # band half width
W = 101            # band window width per row
SW = 104           # S tile width (101 + 3 zero guard cols)
NCHUNK = 2         # 128-row chunks per half
WIN = 228          # matmul j-window: 128 + 101 - 1
ACOLS = HN         # a-side cols per sequence
AV = HN + BW       # valid b prefix length (306)
BCOLS = 356        # b cols per sequence: 50 left pad + 306
RS = 8             # rescale cadence (rows)
GROWS = 32         # E-tile group rows
NGRP = HN // GROWS
NEVT = HN // RS    # 32 rescale events
CAP = 1e30
NEG = -20000.0     # pad dot-product value -> exp() == 0


def _build_nc():
    nc = bacc.Bacc("TRN2", target_bir_lowering=False, debug=False)
    xt = nc.dram_tensor("xt", [NBAT, DIM, N], F32, kind="ExternalInput").ap()
    yt = nc.dram_tensor("yt", [NBAT, DIM, N], F32, kind="ExternalInput").ap()
    out = nc.dram_tensor("out", [NPROB, W + 1], F32, kind="ExternalOutput").ap()

    with tile.TileContext(nc) as tc, ExitStack() as ctx:
        _emit(ctx, tc, xt, yt, out)
    nc.compile()
    return nc


def _emit(ctx, tc, xt, yt, out):
    nc = tc.nc

    const = ctx.enter_context(tc.tile_pool(name="const", bufs=1))
    winp = ctx.enter_context(tc.tile_pool(name="winp", bufs=14))
    ps_win = ctx.enter_context(tc.tile_pool(name="ps_win", bufs=1, space="PSUM"))
    ps_misc = ctx.enter_context(tc.tile_pool(name="ps_misc", bufs=1, space="PSUM"))
    dramp = ctx.enter_context(tc.tile_pool(name="dramp", bufs=1, space="DRAM"))
    epool = ctx.enter_context(tc.tile_pool(name="epool", bufs=1))

    scratchP = dramp.tile([NBAT, NCHUNK, 128, 2, WIN], BF16)
    scratchS = dramp.tile([NBAT, NCHUNK, 128, WIN], BF16)
    EPITCH = (GROWS + 1) * W
    e_tiles = [
        epool.tile([NPROB, EPITCH], BF16, name=f"edp{g}", tag=f"edp{g}")
        for g in range(NGRP)
    ]

    QQ = [nc.sync, nc.scalar, nc.gpsimd]

    # DP state tiles are allocated BEFORE the prep pool so they do not
    # reuse its SBUF (which would add a WAR dependency on the whole prep).
    spool = ctx.enter_context(tc.tile_pool(name="spool", bufs=1))
    cbp = ctx.enter_context(tc.tile_pool(name="cbp", bufs=2))
    m_buf = const.tile([NPROB, NEVT], F32)
    SD = 2 * W + 6        # doubled S row (202) + zero guard
    s_ring = [
        spool.tile([NPROB, SD], BF16, tag=f"s{k}", name=f"s{k}") for k in range(3)
    ]
    for s in s_ring:
        nc.vector.memset(s[:], 0.0)
    for g in range(NGRP):
        nc.vector.memset(e_tiles[g][:, 0:W], 1.0)

    # ---- Phase 0+1 fused: K=6 operand prep pipelined with chunk-0 ---------
    # a6 rows: [a0..a3, ones, -|a|^2/2]; b6 rows: [b0..b3, -|b|^2/2 (NEG
    # pads), ones].  G' = a6 . b6 = a.b - |a|^2/2 - |b|^2/2 = -D/2.
    # Norm rows are produced as 6-partition PSUM matmul outputs (rows 0-3
    # zero) and ACT-copied over a6/b6 BEFORE the feature casts land.
    ps_na = ctx.enter_context(tc.tile_pool(name="ps_na", bufs=1, space="PSUM"))
    with tc.tile_pool(name="pre", bufs=1) as pre:
        a6 = pre.tile([5, NSEQ * ACOLS], BF16)
        b6 = pre.tile([5, NSEQ * BCOLS], BF16)
        nhb = pre.tile([4, 5], BF16)
        nhalf_c = pre.tile([4, 1], BF16)
        stag = pre.tile([4, NSEQ * AV], F32)
        sq = pre.tile([4, NSEQ * AV], BF16)
        ones_stg = pre.tile([1, NSEQ * ACOLS], BF16)
        npad = pre.tile([1, NSEQ * BW], BF16)
        na_sb = pre.tile([128, 2 * NSEQ], F32)
        na_ps = ps_na.tile([128, 2 * NSEQ], F32)

        # b-norm contraction: pnb row 4 = -0.5 * sum(sq); a-norm is applied
        # later as a per-partition bias on the exp instead of a K row.
        nc.vector.memset(nhb[:], 0.0)
        nc.vector.memset(nhb[:, 4:5], -0.5)
        nc.vector.memset(nhalf_c[:], -0.5)
        nc.vector.memset(ones_stg[:], 1.0)
        nc.vector.memset(npad[:], NEG)
        # a6 row 4 = ones; b6 row-4 NEG pads (engine can't write partition 4)
        nc.gpsimd.dma_start(a6[4:5, :], ones_stg[:])
        nc.gpsimd.dma_start(
            b6[4:5, :].rearrange("p (s c) -> p s c", c=BCOLS)[:, :, 0:BW],
            npad[:].rearrange("p (s c) -> p s c", c=BW))

        st3 = stag[:].rearrange("p (s c) -> p s c", c=AV)
        sq3 = sq[0:4, :].rearrange("p (s c) -> p s c", c=AV)
        a3 = a6[0:4, :].rearrange("p (s c) -> p s c", c=ACOLS)
        b3 = b6[0:4, :].rearrange("p (s c) -> p s c", c=BCOLS)
        # paired x/y views: [p, 2(grp), 8(batch), cols] — one DVE op covers
        # both of a batch's sequences (fewer fixed-cost instructions)
        st4 = stag[:].rearrange("p (g s c) -> p g s c", s=NBAT, c=AV)
        sq4 = sq[0:4, :].rearrange("p (g s c) -> p g s c", s=NBAT, c=AV)
        a4 = a6[0:4, :].rearrange("p (g s c) -> p g s c", s=NBAT, c=ACOLS)
        b4 = b6[0:4, :].rearrange("p (g s c) -> p g s c", s=NBAT, c=BCOLS)
        na4s = na_sb[:].rearrange("p (g s k) -> p g s k", s=NBAT, k=2)
        na4p = na_ps[:].rearrange("p (g s k) -> p g s k", s=NBAT, k=2)
        xsrc = xt.transpose([1, 0, 2])
        ysrc = yt.transpose([1, 0, 2])
        hb = NBAT // 2
        nc.sync.dma_start(st3[:, 0:hb, :], xsrc[:, 0:hb, 0:AV])
        nc.scalar.dma_start(st3[:, NBAT:NBAT + hb, :], ysrc[:, 0:hb, 0:AV])
        nc.sync.dma_start(st3[:, hb:NBAT, :], xsrc[:, hb:NBAT, 0:AV])
        nc.scalar.dma_start(
            st3[:, NBAT + hb:NSEQ, :], ysrc[:, hb:NBAT, 0:AV])

        pairs = [(b, NBAT + b) for b in range(NBAT)] \
            + [(b, b) for b in range(NBAT)] \
            + [(NBAT + b, NBAT + b) for b in range(NBAT)]
        scP_h = scratchP[:].tensor
        scS_h = scratchS[:].tensor

        ew_pairs = {}
        ew_sing = {}

        def mm_win(pw_slice, pi, c):
            sa, sb = pairs[pi]
            nc.tensor.matmul(
                pw_slice,
                a6[:, sa * ACOLS + c * 128:sa * ACOLS + (c + 1) * 128],
                b6[:, sb * BCOLS + c * 128:sb * BCOLS + c * 128 + WIN],
                start=True, stop=True,
            )

        def emit_pair(pi0, pi1, c):
            # same a-sequence: one exp with shared per-partition bias
            sa = pairs[pi0][0]
            pw = ps_win.tile([128, 2 * WIN], F32, name=f"pw{pi0}_{c}",
                             tag="pw", bufs=4)
            ew = winp.tile([128, 2 * WIN], BF16, name=f"ew{pi0}_{c}",
                           tag="ew", bufs=8)
            mm_win(pw[:, 0:WIN], pi0, c)
            mm_win(pw[:, WIN:2 * WIN], pi1, c)
            ew_pairs[(pi0, c)] = ew
            nc.scalar.activation(ew[:], pw[:], ACTF.Exp,
                                 bias=na_sb[:, 2 * sa + c:2 * sa + c + 1])

        def emit_single(pi, c):
            sa = pairs[pi][0]
            pw = ps_win.tile([128, WIN], F32, name=f"pws{pi}_{c}",
                             tag="pws", bufs=2)
            ew = winp.tile([128, WIN], BF16, name=f"ews{pi}_{c}",
                           tag="ews", bufs=4)
            mm_win(pw[:], pi, c)
            ew_sing[(pi, c)] = ew
            nc.scalar.activation(ew[:], pw[:], ACTF.Exp,
                                 bias=na_sb[:, 2 * sa + c:2 * sa + c + 1])

        # per-seq prep pipeline: mul (DVE) -> 2 norm matmuls (PE) ->
        # 2 ACT copies -> feature casts (DVE); then this batch's chunk-0
        # problems immediately so the E pipeline starts while later
        # sequences are still being prepped.
        for b in range(NBAT):
            nc.vector.tensor_mul(
                sq4[:, :, b, :], st4[:, :, b, :], st4[:, :, b, :])
            for s in (b, NBAT + b):
                pnb = ps_misc.tile([5, AV], F32, name=f"pnb{s}", tag="pnb")
                nc.tensor.matmul(
                    pnb[:], nhb[:], sq[:, s * AV:(s + 1) * AV],
                    start=True, stop=True)
                for cc in range(NCHUNK):
                    nc.tensor.matmul(
                        na_ps[:, 2 * s + cc:2 * s + cc + 1],
                        sq[:, s * AV + cc * 128:s * AV + (cc + 1) * 128],
                        nhalf_c[:], start=True, stop=True)
                nc.scalar.copy(b6[:, s * BCOLS + BW:(s + 1) * BCOLS], pnb[:])
            nc.vector.tensor_copy(na4s[:, :, b, :], na4p[:, :, b, :])
            nc.vector.tensor_copy(b4[:, :, b, BW:BCOLS], st4[:, :, b, :])
            nc.vector.tensor_copy(a4[:, :, b, :], st4[:, :, b, 0:ACOLS])
            nc.vector.memset(b4[:, :, b, 0:BW], 0.0)
            emit_pair(b, NBAT + b, 0)     # xy + xx share bias na[b]
            emit_single(2 * NBAT + b, 0)  # yy, bias na[8+b]

        PB = NCHUNK * 128 * 2 * WIN     # scratchP problem stride
        PCC = 128 * 2 * WIN             # scratchP chunk stride
        SB = NCHUNK * 128 * WIN         # scratchS problem stride

        def emit_slab_writes_gathers(c, slabs):
            # pair-interleaved writes: one DMA covers (xy,xx) of a batch —
            # half the write triggers (each costs ~0.5-0.8us engine time)
            for h0, wrows in slabs:
                for b in range(NBAT):
                    QQ[b % 3].dma_start(
                        scratchP[b, c, h0:h0 + wrows],
                        ew_pairs[(b, c)][h0:h0 + wrows, :].rearrange(
                            "r (k w) -> r k w", w=WIN))
                for b in range(NBAT):
                    QQ[(b + 1) % 3].dma_start(
                        scratchS[b, c, h0:h0 + wrows],
                        ew_sing[(2 * NBAT + b, c)][h0:h0 + wrows, :])
                for h in range(h0 // GROWS, (h0 + wrows) // GROWS):
                    g = c * (128 // GROWS) + h
                    e3g = e_tiles[g][:, W:].rearrange(
                        "p (r t) -> p r t", t=W)
                    for q, (hnd, base, pstride) in enumerate((
                            (scP_h, 0, PB),          # xy: k=0
                            (scP_h, WIN, PB),        # xx: k=1
                            (scS_h, 0, SB))):        # yy
                        rstride = 2 * WIN + 1 if q < 2 else WIN + 1
                        src = bass.AP(
                            hnd,
                            base + c * (PCC if q < 2 else 128 * WIN)
                            + h * GROWS * rstride,
                            [[pstride, 8], [rstride, GROWS], [1, W]],
                        )
                        QQ[q].dma_start(e3g[8 * q:8 * q + 8], src)

        emit_slab_writes_gathers(0, [(0, 32), (32, 96)])
        for b in range(NBAT):
            emit_pair(b, NBAT + b, 1)
            emit_single(2 * NBAT + b, 1)
        emit_slab_writes_gathers(1, [(0, 128)])

    # ---- Phase 3: exp-space row-scan DP (256 rows) ------------------------
    def scan_fused(out_ap, d0_ap, d1_ap):
        # state = (d0[e] + state) * d1[e] chained across the 2-dim free
        # pattern: per t, k=0 adds S_prev[t] (x1), k=1 adds S_prev[t+1]
        # and multiplies by E[t] — the whole row recurrence in ONE scan.
        v = nc.vector
        return v.add_instruction(
            mybir.InstTensorScalarPtr(
                name=v.bass.get_next_instruction_name(),
                is_tensor_tensor_scan=True,
                is_scalar_tensor_tensor=True,
                op0=ALU.add,
                op1=ALU.mult,
                ins=[v.lower_ap(d0_ap), v.lower_ap_or_imm(0.0),
                     v.lower_ap(d1_ap)],
                outs=[v.lower_ap(out_ap)],
            ))

    DPROWS = int(os.environ.get('KROWS', str(HN)))
    for i in range(DPROWS):
        gi, rl = i // GROWS, i % GROWS
        et = e_tiles[gi]
        st = s_ring[i % 3]
        if i == 0:
            ct = cbp.tile([NPROB, W], BF16, tag="c", name=f"c{i}")
            nc.vector.memset(ct[:], 0.0)
            nc.vector.memset(ct[:, BW:BW + 1], 1.0)
            e_row = et[:, W:].rearrange("p (r t) -> p r t", t=W)[:, 0, :]
            st_odd = st[:, 0:2 * W].rearrange(
                "p (t k) -> p t k", k=2)[:, :, 1]
            nc.vector.tensor_tensor_scan(
                st_odd, ct[:], e_row, 0.0, ALU.add, ALU.mult)
        else:
            # rows inside the band's left corner skip dead (E=0) elements
            t0 = BW - i if i < BW else 0
            sp = s_ring[(i - 1) % 3]
            d0 = bass.AP(sp[:].tensor, 1 + 2 * t0,
                         [[SD, NPROB], [2, W - t0], [2, 2]])
            d1 = bass.AP(et[:].tensor, t0,
                         [[EPITCH, NPROB], [1, W - t0], [(rl + 1) * W, 2]])
            scan_fused(st[:, 2 * t0:2 * W], d0, d1)
        if i % RS == RS - 1:
            ev = i // RS
            # m_buf stores 1/m; readout subtracts sum(ln(1/m))
            nc.vector.reciprocal(
                m_buf[:, ev:ev + 1], st[:, 2 * BW + 1:2 * BW + 2])
            so = st[:, 0:2 * W].rearrange("p (t k) -> p t k", k=2)[:, :, 1]
            nc.vector.tensor_scalar(
                so, so, m_buf[:, ev:ev + 1], CAP, ALU.mult, ALU.min
            )

    # ---- Phase 4: readout — boundary row + log-rescale sum ----------------
    ln_m = const.tile([NPROB, NEVT], F32)
    obuf = const.tile([NPROB, W + 1], F32)
    nc.scalar.activation(ln_m[:], m_buf[:], ACTF.Ln)
    nc.vector.reduce_sum(obuf[:, W:W + 1], ln_m[:], axis=mybir.AxisListType.X)
    s_last = s_ring[(DPROWS - 1) % 3 if DPROWS else 0]
    s_odd = s_last[:, 0:2 * W].rearrange("p (t k) -> p t k", k=2)[:, :, 1]
    nc.scalar.copy(obuf[:, 0:W], s_odd)
    nc.sync.dma_start(out, obuf[:])


_NC_CACHE = None


def _get_nc():
    global _NC_CACHE
    if _NC_CACHE is None:
        _NC_CACHE = _build_nc()
    return _NC_CACHE


def _in_maps(x, y):
    """Per-core inputs: cores 0-3 forward batches 8g..8g+7, cores 4-7 the
    same batches with sequences reversed (backward half)."""
    maps = []
    for c in range(NCORE):
        g = c % 4
        xs = x[NBAT * g:NBAT * (g + 1)].transpose(0, 2, 1)
        ys = y[NBAT * g:NBAT * (g + 1)].transpose(0, 2, 1)
        if c >= 4:
            xs = xs[:, :, ::-1]
            ys = ys[:, :, ::-1]
        maps.append({
            "xt": np.ascontiguousarray(xs),
            "yt": np.ascontiguousarray(ys),
        })
    return maps


def _combine(outs):
    """Host combine: S_total = sum_t F[t]*(Bp[101-t] + Bp[100-t]);
    loss = R_xy - (R_xx + R_yy)/2 with R = -2*(ln S - tsumF - tsumB)."""
    loss = np.zeros(NBAT * 4, np.float32)
    for g in range(4):
        Fo = np.asarray(outs[g]).reshape(NPROB, W + 1).astype(np.float64)
        Bo = np.asarray(outs[g + 4]).reshape(NPROB, W + 1).astype(np.float64)
        Frow, lF = Fo[:, 0:W], Fo[:, W]
        Brow, lB = Bo[:, 0:W], Bo[:, W]
        rev = Brow[:, ::-1]
        shift = np.concatenate([np.zeros((NPROB, 1)), rev[:, :-1]], 1)
        S = (Frow * (rev + shift)).sum(1)
        R = -2.0 * (np.log(S) - lF - lB)
        loss[NBAT * g:NBAT * (g + 1)] = (
            R[0:NBAT] - 0.5 * (R[NBAT:2 * NBAT] + R[2 * NBAT:])
        ).astype(np.float32)
    return loss


def kernel(x: np.ndarray, y: np.ndarray) -> np.ndarray:
    x = np.ascontiguousarray(x, np.float32)
    y = np.ascontiguousarray(y, np.float32)
    B = x.shape[0]
    assert x.shape == (B, N, DIM) and B == NBAT * 4
    nc = _get_nc()
    res = run_bass_kernel_spmd(nc, _in_maps(x, y), list(range(NCORE)))
    outs = [res.results[k]["out"] for k in range(NCORE)]
    return _combine(outs)


if __name__ == "__main__":
    xx = np.random.randn(32, N, DIM).astype(np.float32)
    yy = np.random.randn(32, N, DIM).astype(np.float32)
    print(kernel(xx, yy)[:4])


# revision 46
# speedup vs baseline: 1.0016x; 1.0016x over previous
"""Banded soft-DTW loss (normalize=True) Trainium2 Bass kernel.

Problem: x, y [32, 512, 4] f32 -> loss [32] f32
  loss = softdtw(x,y) - 0.5*(softdtw(x,x) + softdtw(y,y)), gamma=2, band=50.

Strategy — forward/backward split (halves the serial DP chain):
  * The soft-DTW path-sum factors at the middle cut:
      S_total = sum_j F[255,j] * (B[256,j] + B[256,j+1])
    where F is the forward exp-space DP after rows 0..255 and B the
    backward DP, which equals the forward DP of the REVERSED sequences.
  * Cores 0-3 run forward halves, cores 4-7 run the same program on
    host-reversed inputs. Each core: 24 DP problems (8 batches x
    {xy,xx,yy}) x 256 rows, batched across 24 SBUF partitions.
  * Cost matrices: one K=5 augmented matmul per (problem, 128-row chunk)
    computes a.b - |b|^2/2 over a 228-wide j-window; the -|a|^2/2 term is
    applied as the exp's per-partition bias (from tiny transposed
    matmuls); (xy, xx) problems share one PSUM tile and one ACT exp.
    DRAM round-trip extracts the 101-wide diagonal band per row, written
    in latency-tiered slabs and gathered into 32-row groups that carry a
    ones-prefix for the DP scan below.
  * DP: ONE fused 202-element tensor_tensor_scan per row (emitted at the
    ISA level; the HW chains scan state across access-pattern dims):
    per t, element (t,0) adds S_prev[t] (times the ones-prefix) and
    (t,1) adds S_prev[t+1] and multiplies by E_i[t] — the full row
    recurrence S_i[t] = E_i[t]*(S_i[t-1] + S_{i-1}[t] + S_{i-1}[t+1])
    with no separate add op. S rows live in doubled layout (odd slots);
    rows < BW start at the band edge. Rescale by the diagonal cell every
    RS rows (log accumulated, applied to odd slots only).
  * Each core outputs its boundary row + log-rescale sum; the host does
    the tiny O(B*W) combine.
"""
import os
import sys
from contextlib import ExitStack

import numpy as np

for _p in ("/opt/trn_rl_repo", "/root/.axon_site/_ro/trn_rl_repo"):
    if os.path.isdir(_p) and _p not in sys.path:
        sys.path.append(_p)

import concourse.bass as bass
import concourse.bacc as bacc
import concourse.mybir as mybir
import concourse.tile as tile
from concourse.bass_utils import run_bass_kernel_spmd

F32 = mybir.dt.float32
BF16 = mybir.dt.bfloat16
ALU = mybir.AluOpType
ACTF = mybir.ActivationFunctionType

N = 512            # full sequence length
HN = 256           # rows per half (per core)
DIM = 4
NBAT = 8           # batch elements per core
NSEQ = 16          # sequences per core (8 x + 8 y)
NPROB = 24         # DP problems per core (xy, xx, yy for 8 batches)
NCORE = 8
BW = 50            # band half width
W = 101            # band window width per row
SW = 104           # S tile width (101 + 3 zero guard cols)
NCHUNK = 2         # 128-row chunks per half
WIN = 228          # matmul j-window: 128 + 101 - 1
ACOLS = HN         # a-side cols per sequence
AV = HN + BW       # valid b prefix length (306)
BCOLS = 356        # b cols per sequence: 50 left pad + 306
RS = 8             # rescale cadence (rows)
GROWS = 32         # E-tile group rows
NGRP = HN // GROWS
NEVT = HN // RS    # 32 rescale events
CAP = 1e30
NEG = -20000.0     # pad dot-product value -> exp() == 0


def _build_nc():
    nc = bacc.Bacc("TRN2", target_bir_lowering=False, debug=False)
    xt = nc.dram_tensor("xt", [NBAT, DIM, N], F32, kind="ExternalInput").ap()
    yt = nc.dram_tensor("yt", [NBAT, DIM, N], F32, kind="ExternalInput").ap()
    out = nc.dram_tensor("out", [NPROB, W + 1], F32, kind="ExternalOutput").ap()

    with tile.TileContext(nc) as tc, ExitStack() as ctx:
        _emit(ctx, tc, xt, yt, out)
    nc.compile()
    return nc


def _emit(ctx, tc, xt, yt, out):
    nc = tc.nc

    const = ctx.enter_context(tc.tile_pool(name="const", bufs=1))
    winp = ctx.enter_context(tc.tile_pool(name="winp", bufs=14))
    ps_win = ctx.enter_context(tc.tile_pool(name="ps_win", bufs=1, space="PSUM"))
    ps_misc = ctx.enter_context(tc.tile_pool(name="ps_misc", bufs=1, space="PSUM"))
    dramp = ctx.enter_context(tc.tile_pool(name="dramp", bufs=1, space="DRAM"))
    epool = ctx.enter_context(tc.tile_pool(name="epool", bufs=1))

    scratch = dramp.tile([NPROB, NCHUNK, 128, WIN], BF16)
    EPITCH = (GROWS + 1) * W
    e_tiles = [
        epool.tile([NPROB, EPITCH], BF16, name=f"edp{g}", tag=f"edp{g}")
        for g in range(NGRP)
    ]

    QQ = [nc.sync, nc.scalar, nc.gpsimd]

    # DP state tiles are allocated BEFORE the prep pool so they do not
    # reuse its SBUF (which would add a WAR dependency on the whole prep).
    spool = ctx.enter_context(tc.tile_pool(name="spool", bufs=1))
    cbp = ctx.enter_context(tc.tile_pool(name="cbp", bufs=2))
    m_buf = const.tile([NPROB, NEVT], F32)
    SD = 2 * W + 6        # doubled S row (202) + zero guard
    s_ring = [
        spool.tile([NPROB, SD], BF16, tag=f"s{k}", name=f"s{k}") for k in range(3)
    ]
    for s in s_ring:
        nc.vector.memset(s[:], 0.0)
    for g in range(NGRP):
        nc.vector.memset(e_tiles[g][:, 0:W], 1.0)

    # ---- Phase 0+1 fused: K=6 operand prep pipelined with chunk-0 ---------
    # a6 rows: [a0..a3, ones, -|a|^2/2]; b6 rows: [b0..b3, -|b|^2/2 (NEG
    # pads), ones].  G' = a6 . b6 = a.b - |a|^2/2 - |b|^2/2 = -D/2.
    # Norm rows are produced as 6-partition PSUM matmul outputs (rows 0-3
    # zero) and ACT-copied over a6/b6 BEFORE the feature casts land.
    ps_na = ctx.enter_context(tc.tile_pool(name="ps_na", bufs=1, space="PSUM"))
    with tc.tile_pool(name="pre", bufs=1) as pre:
        a6 = pre.tile([5, NSEQ * ACOLS], BF16)
        b6 = pre.tile([5, NSEQ * BCOLS], BF16)
        nhb = pre.tile([4, 5], BF16)
        nhalf_c = pre.tile([4, 1], BF16)
        stag = pre.tile([4, NSEQ * AV], F32)
        sq = pre.tile([4, NSEQ * AV], BF16)
        ones_stg = pre.tile([1, NSEQ * ACOLS], BF16)
        npad = pre.tile([1, NSEQ * BW], BF16)
        na_sb = pre.tile([128, 2 * NSEQ], F32)
        na_ps = ps_na.tile([128, 2 * NSEQ], F32)

        # b-norm contraction: pnb row 4 = -0.5 * sum(sq); a-norm is applied
        # later as a per-partition bias on the exp instead of a K row.
        nc.vector.memset(nhb[:], 0.0)
        nc.vector.memset(nhb[:, 4:5], -0.5)
        nc.vector.memset(nhalf_c[:], -0.5)
        nc.vector.memset(ones_stg[:], 1.0)
        nc.vector.memset(npad[:], NEG)
        # a6 row 4 = ones; b6 row-4 NEG pads (engine can't write partition 4)
        nc.gpsimd.dma_start(a6[4:5, :], ones_stg[:])
        nc.gpsimd.dma_start(
            b6[4:5, :].rearrange("p (s c) -> p s c", c=BCOLS)[:, :, 0:BW],
            npad[:].rearrange("p (s c) -> p s c", c=BW))

        st3 = stag[:].rearrange("p (s c) -> p s c", c=AV)
        sq3 = sq[0:4, :].rearrange("p (s c) -> p s c", c=AV)
        a3 = a6[0:4, :].rearrange("p (s c) -> p s c", c=ACOLS)
        b3 = b6[0:4, :].rearrange("p (s c) -> p s c", c=BCOLS)
        # paired x/y views: [p, 2(grp), 8(batch), cols] — one DVE op covers
        # both of a batch's sequences (fewer fixed-cost instructions)
        st4 = stag[:].rearrange("p (g s c) -> p g s c", s=NBAT, c=AV)
        sq4 = sq[0:4, :].rearrange("p (g s c) -> p g s c", s=NBAT, c=AV)
        a4 = a6[0:4, :].rearrange("p (g s c) -> p g s c", s=NBAT, c=ACOLS)
        b4 = b6[0:4, :].rearrange("p (g s c) -> p g s c", s=NBAT, c=BCOLS)
        na4s = na_sb[:].rearrange("p (g s k) -> p g s k", s=NBAT, k=2)
        na4p = na_ps[:].rearrange("p (g s k) -> p g s k", s=NBAT, k=2)
        xsrc = xt.transpose([1, 0, 2])
        ysrc = yt.transpose([1, 0, 2])
        hb = NBAT // 2
        nc.sync.dma_start(st3[:, 0:hb, :], xsrc[:, 0:hb, 0:AV])
        nc.scalar.dma_start(st3[:, NBAT:NBAT + hb, :], ysrc[:, 0:hb, 0:AV])
        nc.sync.dma_start(st3[:, hb:NBAT, :], xsrc[:, hb:NBAT, 0:AV])
        nc.scalar.dma_start(
            st3[:, NBAT + hb:NSEQ, :], ysrc[:, hb:NBAT, 0:AV])

        pairs = [(b, NBAT + b) for b in range(NBAT)] \
            + [(b, b) for b in range(NBAT)] \
            + [(NBAT + b, NBAT + b) for b in range(NBAT)]
        sc_handle = scratch[:].tensor

        ew_tiles = {}

        def mm_win(pw_slice, pi, c):
            sa, sb = pairs[pi]
            nc.tensor.matmul(
                pw_slice,
                a6[:, sa * ACOLS + c * 128:sa * ACOLS + (c + 1) * 128],
                b6[:, sb * BCOLS + c * 128:sb * BCOLS + c * 128 + WIN],
                start=True, stop=True,
            )

        def emit_pair(pi0, pi1, c):
            # same a-sequence: one exp with shared per-partition bias
            sa = pairs[pi0][0]
            pw = ps_win.tile([128, 2 * WIN], F32, name=f"pw{pi0}_{c}",
                             tag="pw", bufs=4)
            ew = winp.tile([128, 2 * WIN], BF16, name=f"ew{pi0}_{c}",
                           tag="ew", bufs=8)
            mm_win(pw[:, 0:WIN], pi0, c)
            mm_win(pw[:, WIN:2 * WIN], pi1, c)
            ew_tiles[(pi0, c)] = ew[:, 0:WIN]
            ew_tiles[(pi1, c)] = ew[:, WIN:2 * WIN]
            nc.scalar.activation(ew[:], pw[:], ACTF.Exp,
                                 bias=na_sb[:, 2 * sa + c:2 * sa + c + 1])

        def emit_single(pi, c):
            sa = pairs[pi][0]
            pw = ps_win.tile([128, WIN], F32, name=f"pws{pi}_{c}",
                             tag="pws", bufs=2)
            ew = winp.tile([128, WIN], BF16, name=f"ews{pi}_{c}",
                           tag="ews", bufs=4)
            mm_win(pw[:], pi, c)
            ew_tiles[(pi, c)] = ew[:]
            nc.scalar.activation(ew[:], pw[:], ACTF.Exp,
                                 bias=na_sb[:, 2 * sa + c:2 * sa + c + 1])

        # per-seq prep pipeline: mul (DVE) -> 2 norm matmuls (PE) ->
        # 2 ACT copies -> feature casts (DVE); then this batch's chunk-0
        # problems immediately so the E pipeline starts while later
        # sequences are still being prepped.
        for b in range(NBAT):
            nc.vector.tensor_mul(
                sq4[:, :, b, :], st4[:, :, b, :], st4[:, :, b, :])
            for s in (b, NBAT + b):
                pnb = ps_misc.tile([5, AV], F32, name=f"pnb{s}", tag="pnb")
                nc.tensor.matmul(
                    pnb[:], nhb[:], sq[:, s * AV:(s + 1) * AV],
                    start=True, stop=True)
                for cc in range(NCHUNK):
                    nc.tensor.matmul(
                        na_ps[:, 2 * s + cc:2 * s + cc + 1],
                        sq[:, s * AV + cc * 128:s * AV + (cc + 1) * 128],
                        nhalf_c[:], start=True, stop=True)
                nc.scalar.copy(b6[:, s * BCOLS + BW:(s + 1) * BCOLS], pnb[:])
            nc.vector.tensor_copy(na4s[:, :, b, :], na4p[:, :, b, :])
            nc.vector.tensor_copy(b4[:, :, b, BW:BCOLS], st4[:, :, b, :])
            nc.vector.tensor_copy(a4[:, :, b, :], st4[:, :, b, 0:ACOLS])
            nc.vector.memset(b4[:, :, b, 0:BW], 0.0)
            emit_pair(b, NBAT + b, 0)     # xy + xx share bias na[b]
            emit_single(2 * NBAT + b, 0)  # yy, bias na[8+b]

        def emit_slab_writes_gathers(c, slabs):
            # early rows in fine slabs so the first gather groups unblock
            # fast; later rows coarse (each DMA trigger costs ~0.5-0.8us
            # of engine time).
            for h0, wrows in slabs:
                for pi in range(NPROB):
                    QQ[pi % 3].dma_start(
                        scratch[pi, c, h0:h0 + wrows],
                        ew_tiles[(pi, c)][h0:h0 + wrows, :])
                for h in range(h0 // GROWS, (h0 + wrows) // GROWS):
                    g = c * (128 // GROWS) + h
                    e3g = e_tiles[g][:, W:].rearrange(
                        "p (r t) -> p r t", t=W)
                    for q in range(3):
                        p0 = 8 * q
                        src = bass.AP(
                            sc_handle,
                            p0 * (NCHUNK * 128 * WIN) + c * 128 * WIN
                            + h * GROWS * (WIN + 1),
                            [[NCHUNK * 128 * WIN, 8], [WIN + 1, GROWS],
                             [1, W]],
                        )
                        QQ[q].dma_start(e3g[p0:p0 + 8], src)

        emit_slab_writes_gathers(0, [(0, 32), (32, 96)])
        for b in range(NBAT):
            emit_pair(b, NBAT + b, 1)
            emit_single(2 * NBAT + b, 1)
        emit_slab_writes_gathers(1, [(0, 128)])

    # ---- Phase 3: exp-space row-scan DP (256 rows) ------------------------
    def scan_fused(out_ap, d0_ap, d1_ap):
        # state = (d0[e] + state) * d1[e] chained across the 2-dim free
        # pattern: per t, k=0 adds S_prev[t] (x1), k=1 adds S_prev[t+1]
        # and multiplies by E[t] — the whole row recurrence in ONE scan.
        v = nc.vector
        return v.add_instruction(
            mybir.InstTensorScalarPtr(
                name=v.bass.get_next_instruction_name(),
                is_tensor_tensor_scan=True,
                is_scalar_tensor_tensor=True,
                op0=ALU.add,
                op1=ALU.mult,
                ins=[v.lower_ap(d0_ap), v.lower_ap_or_imm(0.0),
                     v.lower_ap(d1_ap)],
                outs=[v.lower_ap(out_ap)],
            ))

    DPROWS = int(os.environ.get('KROWS', str(HN)))
    for i in range(DPROWS):
        gi, rl = i // GROWS, i % GROWS
        et = e_tiles[gi]
        st = s_ring[i % 3]
        if i == 0:
            ct = cbp.tile([NPROB, W], BF16, tag="c", name=f"c{i}")
            nc.vector.memset(ct[:], 0.0)
            nc.vector.memset(ct[:, BW:BW + 1], 1.0)
            e_row = et[:, W:].rearrange("p (r t) -> p r t", t=W)[:, 0, :]
            st_odd = st[:, 0:2 * W].rearrange(
                "p (t k) -> p t k", k=2)[:, :, 1]
            nc.vector.tensor_tensor_scan(
                st_odd, ct[:], e_row, 0.0, ALU.add, ALU.mult)
        else:
            # rows inside the band's left corner skip dead (E=0) elements
            t0 = BW - i if i < BW else 0
            sp = s_ring[(i - 1) % 3]
            d0 = bass.AP(sp[:].tensor, 1 + 2 * t0,
                         [[SD, NPROB], [2, W - t0], [2, 2]])
            d1 = bass.AP(et[:].tensor, t0,
                         [[EPITCH, NPROB], [1, W - t0], [(rl + 1) * W, 2]])
            scan_fused(st[:, 2 * t0:2 * W], d0, d1)
        if i % RS == RS - 1:
            ev = i // RS
            # m_buf stores 1/m; readout subtracts sum(ln(1/m))
            nc.vector.reciprocal(
                m_buf[:, ev:ev + 1], st[:, 2 * BW + 1:2 * BW + 2])
            so = st[:, 0:2 * W].rearrange("p (t k) -> p t k", k=2)[:, :, 1]
            nc.vector.tensor_scalar(
                so, so, m_buf[:, ev:ev + 1], CAP, ALU.mult, ALU.min
            )

    # ---- Phase 4: readout — boundary row + log-rescale sum ----------------
    ln_m = const.tile([NPROB, NEVT], F32)
    obuf = const.tile([NPROB, W + 1], F32)
    nc.scalar.activation(ln_m[:], m_buf[:], ACTF.Ln)
    nc.vector.reduce_sum(obuf[:, W:W + 1], ln_m[:], axis=mybir.AxisListType.X)
    s_last = s_ring[(DPROWS - 1) % 3 if DPROWS else 0]
    s_odd = s_last[:, 0:2 * W].rearrange("p (t k) -> p t k", k=2)[:, :, 1]
    nc.scalar.copy(obuf[:, 0:W], s_odd)
    nc.sync.dma_start(out, obuf[:])


_NC_CACHE = None


def _get_nc():
    global _NC_CACHE
    if _NC_CACHE is None:
        _NC_CACHE = _build_nc()
    return _NC_CACHE


def _in_maps(x, y):
    """Per-core inputs: cores 0-3 forward batches 8g..8g+7, cores 4-7 the
    same batches with sequences reversed (backward half)."""
    maps = []
    for c in range(NCORE):
        g = c % 4
        xs = x[NBAT * g:NBAT * (g + 1)].transpose(0, 2, 1)
        ys = y[NBAT * g:NBAT * (g + 1)].transpose(0, 2, 1)
        if c >= 4:
            xs = xs[:, :, ::-1]
            ys = ys[:, :, ::-1]
        maps.append({
            "xt": np.ascontiguousarray(xs),
            "yt": np.ascontiguousarray(ys),
        })
    return maps


def _combine(outs):
    """Host combine: S_total = sum_t F[t]*(Bp[101-t] + Bp[100-t]);
    loss = R_xy - (R_xx + R_yy)/2 with R = -2*(ln S - tsumF - tsumB)."""
    loss = np.zeros(NBAT * 4, np.float32)
    for g in range(4):
        Fo = np.asarray(outs[g]).reshape(NPROB, W + 1).astype(np.float64)
        Bo = np.asarray(outs[g + 4]).reshape(NPROB, W + 1).astype(np.float64)
        Frow, lF = Fo[:, 0:W], Fo[:, W]
        Brow, lB = Bo[:, 0:W], Bo[:, W]
        rev = Brow[:, ::-1]
        shift = np.concatenate([np.zeros((NPROB, 1)), rev[:, :-1]], 1)
        S = (Frow * (rev + shift)).sum(1)
        R = -2.0 * (np.log(S) - lF - lB)
        loss[NBAT * g:NBAT * (g + 1)] = (
            R[0:NBAT] - 0.5 * (R[NBAT:2 * NBAT] + R[2 * NBAT:])
        ).astype(np.float32)
    return loss


def kernel(x: np.ndarray, y: np.ndarray) -> np.ndarray:
    x = np.ascontiguousarray(x, np.float32)
    y = np.ascontiguousarray(y, np.float32)
    B = x.shape[0]
    assert x.shape == (B, N, DIM) and B == NBAT * 4
    nc = _get_nc()
    res = run_bass_kernel_spmd(nc, _in_maps(x, y), list(range(NCORE)))
    outs = [res.results[k]["out"] for k in range(NCORE)]
    return _combine(outs)


if __name__ == "__main__":
    xx = np.random.randn(32, N, DIM).astype(np.float32)
    yy = np.random.randn(32, N, DIM).astype(np.float32)
    print(kernel(xx, yy)[:4])


# revision 47
# speedup vs baseline: 1.0192x; 1.0176x over previous
"""Banded soft-DTW loss (normalize=True) Trainium2 Bass kernel.

Problem: x, y [32, 512, 4] f32 -> loss [32] f32
  loss = softdtw(x,y) - 0.5*(softdtw(x,x) + softdtw(y,y)), gamma=2, band=50.

Strategy — forward/backward split (halves the serial DP chain):
  * The soft-DTW path-sum factors at the middle cut:
      S_total = sum_j F[255,j] * (B[256,j] + B[256,j+1])
    where F is the forward exp-space DP after rows 0..255 and B the
    backward DP, which equals the forward DP of the REVERSED sequences.
  * Cores 0-3 run forward halves, cores 4-7 run the same program on
    host-reversed inputs. Each core: 24 DP problems (8 batches x
    {xy,xx,yy}) x 256 rows, batched across 24 SBUF partitions.
  * Cost matrices: one K=5 augmented matmul per (problem, 128-row chunk)
    computes a.b - |b|^2/2 over a 228-wide j-window; the -|a|^2/2 term is
    applied as the exp's per-partition bias (from tiny transposed
    matmuls); (xy, xx) problems share one PSUM tile and one ACT exp.
    DRAM round-trip extracts the 101-wide diagonal band per row, written
    in latency-tiered slabs and gathered into 32-row groups that carry a
    ones-prefix for the DP scan below.
  * DP: ONE fused 202-element tensor_tensor_scan per row (emitted at the
    ISA level; the HW chains scan state across access-pattern dims):
    per t, element (t,0) adds S_prev[t] (times the ones-prefix) and
    (t,1) adds S_prev[t+1] and multiplies by E_i[t] — the full row
    recurrence S_i[t] = E_i[t]*(S_i[t-1] + S_{i-1}[t] + S_{i-1}[t+1])
    with no separate add op. S rows live in doubled layout (odd slots);
    rows < BW start at the band edge. Rescale by the diagonal cell every
    RS rows (log accumulated, applied to odd slots only).
  * Each core outputs its boundary row + log-rescale sum; the host does
    the tiny O(B*W) combine.
"""
import os
import sys
from contextlib import ExitStack

import numpy as np

for _p in ("/opt/trn_rl_repo", "/root/.axon_site/_ro/trn_rl_repo"):
    if os.path.isdir(_p) and _p not in sys.path:
        sys.path.append(_p)

import concourse.bass as bass
import concourse.bacc as bacc
import concourse.mybir as mybir
import concourse.tile as tile
from concourse.bass_utils import run_bass_kernel_spmd

F32 = mybir.dt.float32
BF16 = mybir.dt.bfloat16
ALU = mybir.AluOpType
ACTF = mybir.ActivationFunctionType

N = 512            # full sequence length
HN = 256           # rows per half (per core)
DIM = 4
NBAT = 8           # batch elements per core
NSEQ = 16          # sequences per core (8 x + 8 y)
NPROB = 24         # DP problems per core (xy, xx, yy for 8 batches)
NCORE = 8
BW = 50            # band half width
W = 101            # band window width per row
SW = 104           # S tile width (101 + 3 zero guard cols)
NCHUNK = 2         # 128-row chunks per half
WIN = 228          # matmul j-window: 128 + 101 - 1
ACOLS = HN         # a-side cols per sequence
AV = HN + BW       # valid b prefix length (306)
BCOLS = 356        # b cols per sequence: 50 left pad + 306
RS = 8             # rescale cadence (rows)
GROWS = 32         # E-tile group rows
NGRP = HN // GROWS
NEVT = HN // RS    # 32 rescale events
CAP = 1e30
NEG = -20000.0     # pad dot-product value -> exp() == 0


def _build_nc():
    nc = bacc.Bacc("TRN2", target_bir_lowering=False, debug=False)
    xt = nc.dram_tensor("xt", [NBAT, DIM, N], F32, kind="ExternalInput").ap()
    yt = nc.dram_tensor("yt", [NBAT, DIM, N], F32, kind="ExternalInput").ap()
    out = nc.dram_tensor("out", [NPROB, W + 1], F32, kind="ExternalOutput").ap()

    with tile.TileContext(nc) as tc, ExitStack() as ctx:
        _emit(ctx, tc, xt, yt, out)
    nc.compile()
    return nc


def _emit(ctx, tc, xt, yt, out):
    nc = tc.nc

    const = ctx.enter_context(tc.tile_pool(name="const", bufs=1))
    winp = ctx.enter_context(tc.tile_pool(name="winp", bufs=14))
    ps_win = ctx.enter_context(tc.tile_pool(name="ps_win", bufs=1, space="PSUM"))
    ps_misc = ctx.enter_context(tc.tile_pool(name="ps_misc", bufs=1, space="PSUM"))
    dramp = ctx.enter_context(tc.tile_pool(name="dramp", bufs=1, space="DRAM"))
    epool = ctx.enter_context(tc.tile_pool(name="epool", bufs=1))

    scratch = dramp.tile([NPROB, NCHUNK, 128, WIN], BF16)
    EPITCH = (GROWS + 1) * W
    e_tiles = [
        epool.tile([NPROB, EPITCH], BF16, name=f"edp{g}", tag=f"edp{g}")
        for g in range(NGRP)
    ]

    QQ = [nc.sync, nc.scalar, nc.gpsimd]

    # DP state tiles are allocated BEFORE the prep pool so they do not
    # reuse its SBUF (which would add a WAR dependency on the whole prep).
    spool = ctx.enter_context(tc.tile_pool(name="spool", bufs=1))
    cbp = ctx.enter_context(tc.tile_pool(name="cbp", bufs=2))
    m_buf = const.tile([NPROB, NEVT], F32)
    SD = 2 * W + 6        # doubled S row (202) + zero guard
    s_ring = [
        spool.tile([NPROB, SD], BF16, tag=f"s{k}", name=f"s{k}") for k in range(3)
    ]
    for s in s_ring:
        nc.vector.memset(s[:], 0.0)
    for g in range(NGRP):
        nc.vector.memset(e_tiles[g][:, 0:W], 1.0)

    # ---- Phase 0+1 fused: K=6 operand prep pipelined with chunk-0 ---------
    # a6 rows: [a0..a3, ones, -|a|^2/2]; b6 rows: [b0..b3, -|b|^2/2 (NEG
    # pads), ones].  G' = a6 . b6 = a.b - |a|^2/2 - |b|^2/2 = -D/2.
    # Norm rows are produced as 6-partition PSUM matmul outputs (rows 0-3
    # zero) and ACT-copied over a6/b6 BEFORE the feature casts land.
    ps_na = ctx.enter_context(tc.tile_pool(name="ps_na", bufs=1, space="PSUM"))
    with tc.tile_pool(name="pre", bufs=1) as pre:
        a6 = pre.tile([5, NSEQ * ACOLS], BF16)
        b6 = pre.tile([5, NSEQ * BCOLS], BF16)
        nhb = pre.tile([4, 5], BF16)
        nhalf_c = pre.tile([4, 1], BF16)
        stag = pre.tile([4, NSEQ * AV], F32)
        sq = pre.tile([4, NSEQ * AV], BF16)
        ones_stg = pre.tile([1, NSEQ * ACOLS], BF16)
        npad = pre.tile([1, NSEQ * BW], BF16)
        na_sb = pre.tile([128, 2 * NSEQ], F32)
        na_ps = ps_na.tile([128, 2 * NSEQ], F32)

        # b-norm contraction: pnb row 4 = -0.5 * sum(sq); a-norm is applied
        # later as a per-partition bias on the exp instead of a K row.
        nc.vector.memset(nhb[:], 0.0)
        nc.vector.memset(nhb[:, 4:5], -0.5)
        nc.vector.memset(nhalf_c[:], -0.5)
        nc.vector.memset(ones_stg[:], 1.0)
        nc.vector.memset(npad[:], NEG)
        # a6 row 4 = ones; b6 row-4 NEG pads (engine can't write partition 4)
        nc.gpsimd.dma_start(a6[4:5, :], ones_stg[:])
        nc.gpsimd.dma_start(
            b6[4:5, :].rearrange("p (s c) -> p s c", c=BCOLS)[:, :, 0:BW],
            npad[:].rearrange("p (s c) -> p s c", c=BW))

        st3 = stag[:].rearrange("p (s c) -> p s c", c=AV)
        sq3 = sq[0:4, :].rearrange("p (s c) -> p s c", c=AV)
        a3 = a6[0:4, :].rearrange("p (s c) -> p s c", c=ACOLS)
        b3 = b6[0:4, :].rearrange("p (s c) -> p s c", c=BCOLS)
        # paired x/y views: [p, 2(grp), 8(batch), cols] — one DVE op covers
        # both of a batch's sequences (fewer fixed-cost instructions)
        st4 = stag[:].rearrange("p (g s c) -> p g s c", s=NBAT, c=AV)
        sq4 = sq[0:4, :].rearrange("p (g s c) -> p g s c", s=NBAT, c=AV)
        a4 = a6[0:4, :].rearrange("p (g s c) -> p g s c", s=NBAT, c=ACOLS)
        b4 = b6[0:4, :].rearrange("p (g s c) -> p g s c", s=NBAT, c=BCOLS)
        na4s = na_sb[:].rearrange("p (g s k) -> p g s k", s=NBAT, k=2)
        na4p = na_ps[:].rearrange("p (g s k) -> p g s k", s=NBAT, k=2)
        xsrc = xt.transpose([1, 0, 2])
        ysrc = yt.transpose([1, 0, 2])
        hb = NBAT // 2
        nc.sync.dma_start(st3[:, 0:hb, :], xsrc[:, 0:hb, 0:AV])
        nc.scalar.dma_start(st3[:, NBAT:NBAT + hb, :], ysrc[:, 0:hb, 0:AV])
        nc.sync.dma_start(st3[:, hb:NBAT, :], xsrc[:, hb:NBAT, 0:AV])
        nc.scalar.dma_start(
            st3[:, NBAT + hb:NSEQ, :], ysrc[:, hb:NBAT, 0:AV])

        pairs = [(b, NBAT + b) for b in range(NBAT)] \
            + [(b, b) for b in range(NBAT)] \
            + [(NBAT + b, NBAT + b) for b in range(NBAT)]
        sc_handle = scratch[:].tensor

        ew_tiles = {}

        def mm_win(pw_slice, pi, c):
            sa, sb = pairs[pi]
            nc.tensor.matmul(
                pw_slice,
                a6[:, sa * ACOLS + c * 128:sa * ACOLS + (c + 1) * 128],
                b6[:, sb * BCOLS + c * 128:sb * BCOLS + c * 128 + WIN],
                start=True, stop=True,
            )

        def emit_pair(pi0, pi1, c):
            # same a-sequence: one exp with shared per-partition bias
            sa = pairs[pi0][0]
            pw = ps_win.tile([128, 2 * WIN], F32, name=f"pw{pi0}_{c}",
                             tag="pw", bufs=4)
            ew = winp.tile([128, 2 * WIN], BF16, name=f"ew{pi0}_{c}",
                           tag="ew", bufs=8)
            mm_win(pw[:, 0:WIN], pi0, c)
            mm_win(pw[:, WIN:2 * WIN], pi1, c)
            ew_tiles[(pi0, c)] = ew[:, 0:WIN]
            ew_tiles[(pi1, c)] = ew[:, WIN:2 * WIN]
            nc.scalar.activation(ew[:], pw[:], ACTF.Exp,
                                 bias=na_sb[:, 2 * sa + c:2 * sa + c + 1])

        def emit_single(pi, c):
            sa = pairs[pi][0]
            pw = ps_win.tile([128, WIN], F32, name=f"pws{pi}_{c}",
                             tag="pws", bufs=2)
            ew = winp.tile([128, WIN], BF16, name=f"ews{pi}_{c}",
                           tag="ews", bufs=4)
            mm_win(pw[:], pi, c)
            ew_tiles[(pi, c)] = ew[:]
            nc.scalar.activation(ew[:], pw[:], ACTF.Exp,
                                 bias=na_sb[:, 2 * sa + c:2 * sa + c + 1])

        # per-seq prep pipeline: mul (DVE) -> 2 norm matmuls (PE) ->
        # 2 ACT copies -> feature casts (DVE); then this batch's chunk-0
        # problems immediately so the E pipeline starts while later
        # sequences are still being prepped.
        for b in range(NBAT):
            nc.vector.tensor_mul(
                sq4[:, :, b, :], st4[:, :, b, :], st4[:, :, b, :])
            for s in (b, NBAT + b):
                pnb = ps_misc.tile([5, AV], F32, name=f"pnb{s}", tag="pnb")
                nc.tensor.matmul(
                    pnb[:], nhb[:], sq[:, s * AV:(s + 1) * AV],
                    start=True, stop=True)
                for cc in range(NCHUNK):
                    nc.tensor.matmul(
                        na_ps[:, 2 * s + cc:2 * s + cc + 1],
                        sq[:, s * AV + cc * 128:s * AV + (cc + 1) * 128],
                        nhalf_c[:], start=True, stop=True)
                nc.scalar.copy(b6[:, s * BCOLS + BW:(s + 1) * BCOLS], pnb[:])
            nc.vector.tensor_copy(na4s[:, :, b, :], na4p[:, :, b, :])
            nc.vector.tensor_copy(b4[:, :, b, BW:BCOLS], st4[:, :, b, :])
            nc.vector.tensor_copy(a4[:, :, b, :], st4[:, :, b, 0:ACOLS])
            nc.vector.memset(b4[:, :, b, 0:BW], 0.0)
            emit_pair(b, NBAT + b, 0)     # xy + xx share bias na[b]
            emit_single(2 * NBAT + b, 0)  # yy, bias na[8+b]

        def emit_slab_writes_gathers(c, slabs):
            # early rows in fine slabs so the first gather groups unblock
            # fast; later rows coarse (each DMA trigger costs ~0.5-0.8us
            # of engine time).
            for h0, wrows in slabs:
                for pi in range(NPROB):
                    QQ[pi % 3].dma_start(
                        scratch[pi, c, h0:h0 + wrows],
                        ew_tiles[(pi, c)][h0:h0 + wrows, :])
                for h in range(h0 // GROWS, (h0 + wrows) // GROWS):
                    g = c * (128 // GROWS) + h
                    e3g = e_tiles[g][:, W:].rearrange(
                        "p (r t) -> p r t", t=W)
                    for q in range(3):
                        p0 = 8 * q
                        src = bass.AP(
                            sc_handle,
                            p0 * (NCHUNK * 128 * WIN) + c * 128 * WIN
                            + h * GROWS * (WIN + 1),
                            [[NCHUNK * 128 * WIN, 8], [WIN + 1, GROWS],
                             [1, W]],
                        )
                        QQ[q].dma_start(e3g[p0:p0 + 8], src)

        emit_slab_writes_gathers(0, [(0, 32), (32, 96)])
        for b in range(NBAT):
            emit_pair(b, NBAT + b, 1)
            emit_single(2 * NBAT + b, 1)
        emit_slab_writes_gathers(1, [(0, 128)])

    # ---- Phase 3: exp-space row-scan DP (256 rows) ------------------------
    def scan_fused(out_ap, d0_ap, d1_ap):
        # state = (d0[e] + state) * d1[e] chained across the 2-dim free
        # pattern: per t, k=0 adds S_prev[t] (x1), k=1 adds S_prev[t+1]
        # and multiplies by E[t] — the whole row recurrence in ONE scan.
        v = nc.vector
        return v.add_instruction(
            mybir.InstTensorScalarPtr(
                name=v.bass.get_next_instruction_name(),
                is_tensor_tensor_scan=True,
                is_scalar_tensor_tensor=True,
                op0=ALU.add,
                op1=ALU.mult,
                ins=[v.lower_ap(d0_ap), v.lower_ap_or_imm(0.0),
                     v.lower_ap(d1_ap)],
                outs=[v.lower_ap(out_ap)],
            ))

    DPROWS = int(os.environ.get('KROWS', str(HN)))
    for i in range(DPROWS):
        gi, rl = i // GROWS, i % GROWS
        et = e_tiles[gi]
        st = s_ring[i % 3]
        if i == 0:
            ct = cbp.tile([NPROB, W], BF16, tag="c", name=f"c{i}")
            nc.vector.memset(ct[:], 0.0)
            nc.vector.memset(ct[:, BW:BW + 1], 1.0)
            e_row = et[:, W:].rearrange("p (r t) -> p r t", t=W)[:, 0, :]
            st_odd = st[:, 0:2 * W].rearrange(
                "p (t k) -> p t k", k=2)[:, :, 1]
            nc.vector.tensor_tensor_scan(
                st_odd, ct[:], e_row, 0.0, ALU.add, ALU.mult)
        else:
            # rows inside the band's left corner skip dead (E=0) elements
            t0 = BW - i if i < BW else 0
            sp = s_ring[(i - 1) % 3]
            d0 = bass.AP(sp[:].tensor, 1 + 2 * t0,
                         [[SD, NPROB], [2, W - t0], [2, 2]])
            d1 = bass.AP(et[:].tensor, t0,
                         [[EPITCH, NPROB], [1, W - t0], [(rl + 1) * W, 2]])
            scan_fused(st[:, 2 * t0:2 * W], d0, d1)
        if i % RS == RS - 3:
            # stale-diag factor: computed 2 rows early so its latency hides
            # behind an independent scan; any positive factor is exact
            # (logged for readout), only the range cushion shifts ~e^5
            ev = i // RS
            nc.vector.reciprocal(
                m_buf[:, ev:ev + 1], st[:, 2 * BW + 1:2 * BW + 2])
        if i % RS == RS - 1:
            ev = i // RS
            # m_buf stores 1/m; readout subtracts sum(ln(1/m))
            so = st[:, 0:2 * W].rearrange("p (t k) -> p t k", k=2)[:, :, 1]
            nc.vector.tensor_scalar(
                so, so, m_buf[:, ev:ev + 1], CAP, ALU.mult, ALU.min
            )

    # ---- Phase 4: readout — boundary row + log-rescale sum ----------------
    ln_m = const.tile([NPROB, NEVT], F32)
    obuf = const.tile([NPROB, W + 1], F32)
    nc.scalar.activation(ln_m[:], m_buf[:], ACTF.Ln)
    nc.vector.reduce_sum(obuf[:, W:W + 1], ln_m[:], axis=mybir.AxisListType.X)
    s_last = s_ring[(DPROWS - 1) % 3 if DPROWS else 0]
    s_odd = s_last[:, 0:2 * W].rearrange("p (t k) -> p t k", k=2)[:, :, 1]
    nc.scalar.copy(obuf[:, 0:W], s_odd)
    nc.sync.dma_start(out, obuf[:])


_NC_CACHE = None


def _get_nc():
    global _NC_CACHE
    if _NC_CACHE is None:
        _NC_CACHE = _build_nc()
    return _NC_CACHE


def _in_maps(x, y):
    """Per-core inputs: cores 0-3 forward batches 8g..8g+7, cores 4-7 the
    same batches with sequences reversed (backward half)."""
    maps = []
    for c in range(NCORE):
        g = c % 4
        xs = x[NBAT * g:NBAT * (g + 1)].transpose(0, 2, 1)
        ys = y[NBAT * g:NBAT * (g + 1)].transpose(0, 2, 1)
        if c >= 4:
            xs = xs[:, :, ::-1]
            ys = ys[:, :, ::-1]
        maps.append({
            "xt": np.ascontiguousarray(xs),
            "yt": np.ascontiguousarray(ys),
        })
    return maps


def _combine(outs):
    """Host combine: S_total = sum_t F[t]*(Bp[101-t] + Bp[100-t]);
    loss = R_xy - (R_xx + R_yy)/2 with R = -2*(ln S - tsumF - tsumB)."""
    loss = np.zeros(NBAT * 4, np.float32)
    for g in range(4):
        Fo = np.asarray(outs[g]).reshape(NPROB, W + 1).astype(np.float64)
        Bo = np.asarray(outs[g + 4]).reshape(NPROB, W + 1).astype(np.float64)
        Frow, lF = Fo[:, 0:W], Fo[:, W]
        Brow, lB = Bo[:, 0:W], Bo[:, W]
        rev = Brow[:, ::-1]
        shift = np.concatenate([np.zeros((NPROB, 1)), rev[:, :-1]], 1)
        S = (Frow * (rev + shift)).sum(1)
        R = -2.0 * (np.log(S) - lF - lB)
        loss[NBAT * g:NBAT * (g + 1)] = (
            R[0:NBAT] - 0.5 * (R[NBAT:2 * NBAT] + R[2 * NBAT:])
        ).astype(np.float32)
    return loss


def kernel(x: np.ndarray, y: np.ndarray) -> np.ndarray:
    x = np.ascontiguousarray(x, np.float32)
    y = np.ascontiguousarray(y, np.float32)
    B = x.shape[0]
    assert x.shape == (B, N, DIM) and B == NBAT * 4
    nc = _get_nc()
    res = run_bass_kernel_spmd(nc, _in_maps(x, y), list(range(NCORE)))
    outs = [res.results[k]["out"] for k in range(NCORE)]
    return _combine(outs)


if __name__ == "__main__":
    xx = np.random.randn(32, N, DIM).astype(np.float32)
    yy = np.random.randn(32, N, DIM).astype(np.float32)
    print(kernel(xx, yy)[:4])
